# revision 67
# baseline (speedup 1.0000x reference)
"""Trainium2 Bass kernel for nn_EnergyBalanceChecker (segment_reduce), v5.

Problem (hardcoded): B=4, N=512, T=24, G=32, TOL=0.05, EPS=1e-6.

  M = onehot(lv_group_ids) * valid_lv_mask                     # [G, N]
  gc  = einsum('gn,bnt->bgt', M, consumption)
  gg  = einsum('gn,bnt->bgt', M, generation)
  net = einsum('gn,bnt->bgt', M, S.sum(axis=2) - S.sum(axis=1))
  pen = relu(|gc-gg+net| / (gc+gg+eps) - TOL);  out = pen.sum()*w/n_unique

Sharding: 8 cores = 4 batches x 2 halves of the (row) N axis.

v5 dataflow (vs v4): the q-axis fold moves INTO the matmul pass, so there
is no wide-PSUM drain at all, and the output leaves via a pre-prepared
SWDGE scatter fired by a trigger instruction:
  * S streams in fp8e4 as before (SWDGE cast DMAs; cost is charged on
    destination bytes).  Partitions carry (na in 8 n-rows) x (msub in 16
    m-blocks); free = (q, t).
  * Row term: per (block-pair, q) a DoubleRow matmul with the M[g,n]
    projection lhsT and a T-wide rhs slice accumulates straight into a
    single [G, T] PSUM tile (q and pairs both fold in PSUM).
  * Col term: per (block, q-pair) a DoubleRow matmul pairs two q slices
    of one block with per-q lhsT columns -M[g, m(msub,q)] -- accumulating
    *negated* imports into the SAME [G, T] tile, so net = row - col needs
    no subtract, just one PSUM->SBUF copy at the end.
  * gc|gg from a small f16 side input (one matmul pair, mid-stream).
  * Output: a prepare_only dma_scatter_add writes descriptors during the
    stream; a Pool trigger_dma fires them once the [G,4,T] staging tile is
    ready -- skipping the whole HWDGE SEQ/gen/delay chain at the tail.
  * Host does only the [3, G, T]-level nonlinear tail.
"""

import sys

import numpy as np

try:
    import concourse  # noqa: F401
except ImportError:
    sys.path.insert(0, "/opt/trn_rl_repo")

import ml_dtypes

import concourse.tile as tile
from concourse import bacc, mybir
from concourse.bass_utils import run_bass_kernel_spmd

B, N, T, G = 4, 512, 24, 32
TOL, EPS = 0.05, 1e-6
P = 128                 # SBUF partitions
NLOC = N // 2           # rows per core (n-half)
A = 8                   # n-rows per block (partition sub-dim)
C = 16                  # m-blocks on partitions (partition sub-dim)
Q = N // C              # m-columns per msub block (free dim)
QP = Q // 2             # q-pairs for the col-term matmuls
NBLK = NLOC // A        # 32 blocks of 8 n-rows
PAIRS = NBLK // 2       # DoubleRow pairs
F = Q * T               # free elements per block
DMA_BLOCKS = ((0, 8), (8, 13), (13, 19), (19, 24), (24, 28),
              (28, 30), (30, 32))   # stream DMA block ranges; the first is
                                    # large so transfers cover gen latency
SMW = G + 2 * T         # smt row: [mt | cons | gen] per n
BLOB_ROWL = NBLK * G                   # 1024 u8 per partition
BLOB_SMT = 2 * SMW * 2                 # 320 u8 (f16) per partition
BLOBW = BLOB_ROWL + BLOB_SMT
CNW = QP * 2 * G                       # 1024 u8 colN payload (16 partitions)
CNSW = CNW + P                         # + 128 u8 replicate indicator
OW = 3 * T              # out row: [gc | gg | net] per group

_F32 = mybir.dt.float32
_F16 = mybir.dt.float16
_F8 = mybir.dt.float8e4
_U8 = mybir.dt.uint8
_I16 = mybir.dt.int16


def _build_program():
    nc = bacc.Bacc("TRN2", target_bir_lowering=False, debug=False,
                   enable_asserts=False, num_devices=8)
    s = nc.dram_tensor("s", [NLOC, N, T], _F32, kind="ExternalInput").ap()
    blob = nc.dram_tensor("blob", [P, BLOBW], _U8, kind="ExternalInput").ap()
    cns = nc.dram_tensor("cns", [C, CNSW], _U8, kind="ExternalInput").ap()
    # kv_writeback layout: [batch, d_head_inner, d_head_outer, n_ctx]
    out = nc.dram_tensor("out", [1, P, 1, OW], _F32, kind="ExternalOutput").ap()

    with tile.TileContext(nc) as tc, nc.allow_low_precision(
            "fp8 S stream + fp8 {0,1} masks, f32 PSUM accumulation"):
        with (
            tc.tile_pool(name="sb", bufs=1) as sb,
            tc.tile_pool(name="ps", bufs=1, space="PSUM") as ps,
        ):
            blobt = sb.tile([P, BLOBW], _U8, tag="blobt")
            stile = sb.tile([P, NBLK, F], _F8, tag="stile")
            # writeback staging: partition p carries [gc_p | gg_p | net_p] as
            # 72 contiguous f32 (partitions 32..127 are zeroed junk the host
            # ignores); kv_writeback streams the whole [128, 72] block out.
            src4 = sb.tile([P, 1, 1, OW], _F32, tag="src")
            src = src4[:, 0, 0]
            ctxi = sb.tile([P, 1], mybir.dt.int32, tag="ctxi")

            # --- small inputs on two SP HWDGE DMAs, hoisted pre-barrier by
            # the post-compile surgery so their transfers (~530ns) run in the
            # SWDGE warm-up window and the stream follows seamlessly.  colN
            # ships once (16 partitions) and is replicated to 128 partitions
            # on-device via an indicator matmul + ACT cast-copies. ---
            cnst = sb.tile([C, CNSW], _U8, tag="cnst")
            colnt = sb.tile([P, CNW], _F8, tag="colnt")
            nc.sync.dma_start(out=blobt, in_=blob)
            nc.sync.dma_start(out=cnst, in_=cns)

            rowLv = blobt[:, 0:BLOB_ROWL].bitcast(_F8).rearrange(
                "p (b g) -> p b g", b=NBLK)
            smtv = blobt[:, BLOB_ROWL:BLOB_ROWL + BLOB_SMT].bitcast(
                _F16).rearrange("p (nb f) -> p nb f", nb=2)
            cnv = cnst[:, 0:CNW].bitcast(_F8)
            indv = cnst[:, CNW:CNSW].bitcast(_F8)
            colNv = colnt.rearrange("p (j k g) -> p j k g", j=QP, k=2)

            # three banks, all at partition base 0
            gcp = ps.tile([G, T], _F32, tag="gcp")
            ggp = ps.tile([G, T], _F32, tag="ggp")
            netp = ps.tile([G, T], _F32, tag="netp")
            repa = ps.tile([P, CNW // 2], _F32, tag="repa")
            repb = ps.tile([P, CNW // 2], _F32, tag="repb")
            nc.vector.memset(src[:], 0.0)
            nc.vector.memset(ctxi[:], 0)

            # --- S stream: fp8 cast DMAs on the SWDGE ring ---
            # partition p = na*C + msub; block blk: n = blk*A + na;
            # free = (q, t) with m = msub*Q + q.
            s_r = s.rearrange("(blk a) (c q) t -> (a c) blk (q t)", a=A, c=C)
            for b0, b1 in DMA_BLOCKS:
                nc.gpsimd.dma_start(
                    out=stile[:, b0:b1, :].rearrange("p b f -> p (b f)"),
                    in_=s_r[:, b0:b1, :])

            # --- output path: descriptors prepared during the stream, the
            # trigger fires them once `src` is fully written ---
            dma_sem = nc.alloc_semaphore("outdma")
            nc.gpsimd.kv_writeback(
                out, src4[:], ctxi[:], prepare_only=True, sem=dma_sem)

            # --- PE pass ---
            # colN replication first: out[pp, f] = cn[pp % 16, f] via the
            # [16, 128] indicator lhsT; ACT cast-copies land it as fp8.
            H = CNW // 2
            nc.tensor.matmul(repa, indv, cnv[:, 0:H],
                             start=True, stop=True, skip_group_check=True)
            nc.tensor.matmul(repb, indv, cnv[:, H:],
                             start=True, stop=True, skip_group_check=True)
            nc.scalar.copy(out=colnt[:, 0:H], in_=repa)
            nc.scalar.copy(out=colnt[:, H:], in_=repb)
            # gc|gg projections next in PE program order: smt arrives with
            # the blob (~2us), well before the first S pair is consumable.
            for nb in range(2):
                nc.tensor.matmul(gcp, smtv[:, nb, 0:G],
                                 smtv[:, nb, G:G + T],
                                 start=(nb == 0), stop=(nb == 1),
                                 skip_group_check=True)
                nc.tensor.matmul(ggp, smtv[:, nb, 0:G],
                                 smtv[:, nb, G + T:],
                                 start=(nb == 0), stop=(nb == 1),
                                 skip_group_check=True)

            # One [G, T] accumulation group over all 1024 DoubleRow matmuls:
            # row term adds M[g,n]-projected q-slices (2 blocks per pass),
            # col term adds -M[g,m]-weighted q-pairs (2 q per pass).
            netw = netp
            for pr in range(PAIRS):
                lhs_row = rowLv[:, 2 * pr:2 * pr + 2, :]
                for q in range(Q):
                    nc.tensor.matmul(
                        netw, lhs_row,
                        stile[:, 2 * pr:2 * pr + 2, q * T:(q + 1) * T],
                        start=(pr == 0 and q == 0), stop=False,
                        perf_mode=mybir.MatmulPerfMode.DoubleRow,
                        skip_group_check=True)
                for blk in (2 * pr, 2 * pr + 1):
                    for j in range(QP):
                        nc.tensor.matmul(
                            netw, colNv[:, j],
                            stile[:, blk, 2 * j * T:(2 * j + 2) * T]
                            .rearrange("p (k t) -> p k t", k=2),
                            start=False,
                            stop=(pr == PAIRS - 1 and blk == 2 * pr + 1
                                  and j == QP - 1),
                            perf_mode=mybir.MatmulPerfMode.DoubleRow,
                            skip_group_check=True)

            # --- stage [gc | gg | net] and fire the scatter ---
            # Partition-preserving ACT copies; only the net copy is on the
            # post-stream critical path.
            act_done = nc.alloc_semaphore("actdone")
            nc.vector.tensor_copy(out=src[0:G, 0:T], in_=gcp)
            nc.vector.tensor_copy(out=src[0:G, T:2 * T], in_=ggp)
            nc.vector.tensor_copy(out=src[0:G, 2 * T:3 * T], in_=netp)
            # Placeholder gate (>=0 so the schedule-time sim sails through);
            # post-compile surgery points it at the tile Activation engine sem
            # (ACT instructions cannot carry a second sync update, and the
            # trigger cannot carry a second wait).
            nc.gpsimd.wait_ge(act_done, 0)
            nc.gpsimd.trigger_dma(count=None)
    nc.compile()
    # Drop the framework's const-tensor memsets: nothing reads them, but they
    # run on the Pool engine ahead of the barrier and delay the first SWDGE
    # descriptor emission of the S stream.
    for blk in nc.m.functions[0].blocks:
        blk.instructions = [
            i for i in blk.instructions
            if not (type(i).__name__ == "InstMemset"
                    and i.outs and "const-" in str(i.outs[0]))
        ]
    # Tile schedules the scatter prep on a DMASW lane and the exit drain
    # waits on that lane's sem, but the descriptor-baked completion sem
    # (on_update[0], hardware increments by 16) is the user sem= kwarg.
    # Point on_update[0] at the orphaned DMASW sem so the DMA engines bump
    # the sem the drain actually waits on.
    fn = nc.m.functions[0]
    updated, waited, prep = set(), {}, None
    for blk in fn.blocks:
        for ins in blk.instructions:
            if type(ins).__name__ == "InstKVWritebackAnt":
                prep = ins
            si = ins.sync_info
            if si is None:
                continue
            for u in si.on_update:
                updated.add(u.id)
            for w in si.on_wait:
                waited[w.id] = w
    orphans = [w for wid, w in waited.items()
               if wid not in updated and (w.ant_name or "").startswith("DMASW")]
    assert prep is not None and len(orphans) == 1, (prep, orphans)
    u0 = prep.sync_info.on_update[0]
    assert u0.ant_name == "outdma", u0
    prep.sync_info.on_update[0] = mybir.SyncUpdate(
        sync_type=u0.sync_type, id=orphans[0].id, ant_name=orphans[0].ant_name,
        update_mode=u0.update_mode, update_value=u0.update_value,
        update_reg=u0.update_reg)
    # The sem-assignment pass drops the trigger's cross-engine RAW waits (it
    # only gates on the prep's Pool tick), so the trigger could fire before
    # the staging copies.  The placeholder wait_ge(actdone) sits right before
    # the trigger on the Pool SEQ; point it at the Activation engine-proc sem
    # with the cumulative tick of the last staging copy.
    # The scheduler can linearize the ACT exit drain (which waits on the
    # writeback's DMASW sem) BEFORE the staging copies on the same engine --
    # circular in strict block order.  Move the copies ahead of any
    # instruction waiting on the orphan sem.
    orphan_id = orphans[0].id
    for blk in fn.blocks:
        insts = blk.instructions
        drain_pos = None
        for i, ins in enumerate(insts):
            si = ins.sync_info
            if si and any(w.id == orphan_id for w in si.on_wait):
                drain_pos = i
                break
        if drain_pos is None:
            continue
        late = [ins for ins in insts[drain_pos:]
                if type(ins).__name__ in ("InstActivation", "InstTensorCopy")]
        if late:
            rest = [ins for ins in insts if ins not in late]
            blk.instructions = (rest[:drain_pos] + late + rest[drain_pos:])
    # The framework's ACT table load lands in the postamble AFTER the exit
    # wait on the writeback sem, adding ~1.3us of pure tail.  Hoist it to the
    # head of the main block so it overlaps the stream (baseline behavior).
    loads = []
    for blk in fn.blocks:
        keep = []
        for ins in blk.instructions:
            if type(ins).__name__ == "InstLoadActFuncSet":
                loads.append(ins)
            else:
                keep.append(ins)
        blk.instructions = keep
    if loads:
        main = fn.blocks[1]
        main.instructions = loads + main.instructions
    # The trigger can carry only one codegen sync wait; point it at the ACT
    # engine sem tick of the last staging copy (the prep's descriptor gen on
    # the Pool engine finishes several microseconds earlier, so dropping the
    # Pool tick wait is safe).  Delete the placeholder gate entirely.
    act_total = 0
    last_src_tick = None
    trig = None
    gate = None
    for blk in fn.blocks:
        for ins in blk.instructions:
            if type(ins).__name__ == "InstTriggerDma":
                trig = ins
            si = ins.sync_info
            if si is None:
                continue
            for w in si.on_wait:
                if w.ant_name == "actdone":
                    gate = ins
            for u in si.on_update:
                if (u.ant_name or "").startswith("DVE_"):
                    act_total += (u.update_value or 1)
                    if type(ins).__name__ == "InstTensorCopy":
                        last_src_tick = (u.id, u.ant_name, act_total)
    assert trig is not None and last_src_tick is not None, (trig, last_src_tick)
    sid, sname, val = last_src_tick
    trig.sync_info.on_wait = [mybir.SyncWait(
        sync_type="semaphore", id=sid, ant_name=sname,
        wait_mode="sem-ge-imm", wait_value=val, wait_reg=None)]
    if gate is not None:
        for blk in fn.blocks:
            blk.instructions = [i for i in blk.instructions if i is not gate]
    # Hoist the blob HWDGE DMA (SP) and the first stream DMA (Pool) ahead of
    # the entry barrier: their descriptors have no dependencies, so the first
    # transfer starts ~1.3us in instead of ~2.2us.
    main = fn.blocks[1]
    hoist = []
    n_sp = n_pool = 0
    keep = []
    for ins in main.instructions:
        if (type(ins).__name__ == "InstDMACopy" and n_sp < 2
                and ins.engine == mybir.EngineType.SP):
            hoist.append(ins)
            n_sp += 1
        elif (type(ins).__name__ == "InstDMACopy" and n_pool < 1
                and ins.engine == mybir.EngineType.Pool):
            hoist.append(ins)
            n_pool += 1
        else:
            keep.append(ins)
    main.instructions = keep
    fn.blocks[0].instructions = hoist + fn.blocks[0].instructions
    # Exit-barrier trim: the writeback completion is already enforced by the
    # per-engine DMASW waits in the exit block; the trailing gather/release
    # barrier rounds only synchronize engine end times.  Drop them so the
    # kernel ends when the last DMASW waiter releases.
    exit_blk = fn.blocks[-1]
    exit_blk.instructions = [
        i for i in exit_blk.instructions
        if not i.name.startswith("barrier_")
    ]
    # Order the SP exit waits so the sems that are satisfied early (HWDGE,
    # PE, ACT ticks) clear before the late writeback sem wait.
    insts = exit_blk.instructions
    ev = [i for i in insts if type(i).__name__ == "InstEventSemaphore"]
    rest_pos = [j for j, i in enumerate(insts)
                if type(i).__name__ == "InstEventSemaphore"]
    late = [i for i in ev if i.sync_info and any(
        (w.ant_name or "").startswith("DMASW7") for w in i.sync_info.on_wait)]
    early = [i for i in ev if i not in late]
    reordered = early + late
    for j, pos in enumerate(rest_pos):
        insts[pos] = reordered[j]
    exit_blk.instructions = insts
    return nc


_NC_CACHE = None


def _get_program():
    global _NC_CACHE
    if _NC_CACHE is None:
        _NC_CACHE = _build_program()
    return _NC_CACHE


_RUNNER_CACHE = None


def _get_runner():
    """Compiled-once jit(shard_map) executor over 8 cores."""
    global _RUNNER_CACHE
    if _RUNNER_CACHE is None:
        import jax
        from jax.sharding import Mesh, PartitionSpec
        from jax.experimental.shard_map import shard_map
        from concourse import bass2jax, mybir as mb

        nc = _get_program()
        bass2jax.install_neuronx_cc_hook()
        partition_name = (nc.partition_id_tensor.name
                          if nc.partition_id_tensor else None)
        in_names, out_names, out_avals = [], [], []
        for alloc in nc.m.functions[0].allocations:
            if not isinstance(alloc, mb.MemoryLocationSet):
                continue
            name = alloc.memorylocations[0].name
            if alloc.kind == "ExternalInput":
                if name != partition_name:
                    in_names.append(name)
            elif alloc.kind == "ExternalOutput":
                out_names.append(name)
                out_avals.append(jax.core.ShapedArray(
                    tuple(alloc.tensor_shape), mb.dt.np(alloc.dtype)))
        n_params = len(in_names)
        all_names = in_names + out_names
        if partition_name is not None:
            all_names = all_names + [partition_name]

        def _body(*args):
            operands = list(args)
            if partition_name is not None:
                operands.append(bass2jax.partition_id_tensor())
            outs = bass2jax._bass_exec_p.bind(
                *operands,
                out_avals=tuple(out_avals),
                in_names=tuple(all_names),
                out_names=tuple(out_names),
                lowering_input_output_aliases=(),
                sim_require_finite=True,
                sim_require_nnan=True,
                nc=nc,
            )
            return tuple(outs)

        devices = jax.devices()[:8]
        mesh = Mesh(np.asarray(devices), ("core",))
        n_outs = len(out_names)
        sharded = jax.jit(
            shard_map(_body, mesh=mesh,
                      in_specs=(PartitionSpec("core"),) * (n_params + n_outs),
                      out_specs=(PartitionSpec("core"),) * n_outs,
                      check_rep=False),
            donate_argnums=tuple(range(n_params, n_params + n_outs)),
            keep_unused=True,
        )
        _RUNNER_CACHE = (sharded, in_names[:n_params], out_names, out_avals)
    return _RUNNER_CACHE


def _host_side(consumption, generation, sharing_matrix, lv_group_ids,
               valid_lv_mask):
    """Shared input prep: per-core input maps."""
    consumption = np.ascontiguousarray(consumption, dtype=np.float32)
    generation = np.ascontiguousarray(generation, dtype=np.float32)
    sharing_matrix = np.ascontiguousarray(sharing_matrix, dtype=np.float32)
    ids = np.asarray(lv_group_ids)
    valid = np.asarray(valid_lv_mask, dtype=np.float32)

    onehot = (ids[None, :] == np.arange(G)[:, None]).astype(np.float32)
    n_unique = np.float32(np.unique(ids).size)
    M = onehot * valid[None, :]                      # [G, N]
    mt = np.ascontiguousarray(M.T)                   # [N, G]

    # cns[msub, j, k, g] = -M[g, msub*Q + 2j + k]  (negated imports), plus
    # the [16, 128] replicate indicator ind[p, pp] = (pp % 16 == p).
    cw = (-mt).reshape(C, Q * G)                     # [msub, (q, g)]
    cn16 = cw.astype(ml_dtypes.float8_e4m3).view(np.uint8)
    ind = (np.arange(P)[None, :] % C == np.arange(C)[:, None]).astype(
        np.float32).astype(ml_dtypes.float8_e4m3).view(np.uint8)
    cns = np.ascontiguousarray(
        np.concatenate([cn16, ind], axis=1))         # [16, CNSW]

    in_maps = []
    for c in range(8):
        b, hh = divmod(c, 2)
        sl = slice(hh * NLOC, (hh + 1) * NLOC)
        mt_half = mt[sl]                             # [NLOC, G]
        # rowL[p=(na,msub), blk, g] = M[g, blk*A + na]
        proj = mt_half.reshape(NBLK, A, G).transpose(1, 0, 2)   # [na, blk, g]
        proj = np.broadcast_to(proj[:, None], (A, C, NBLK, G))
        rowl8 = proj.reshape(P, NBLK * G).astype(
            ml_dtypes.float8_e4m3).view(np.uint8)
        # smt[p, nb, f]: f = [mt row | cons row | gen row] for n = nb*P + p
        sm = np.empty((2, P, SMW), np.float16)
        sm[:, :, :G] = mt_half.reshape(2, P, G)
        sm[:, :, G:G + T] = consumption[b, sl].reshape(2, P, T)
        sm[:, :, G + T:] = generation[b, sl].reshape(2, P, T)
        sm_bytes = sm.transpose(1, 0, 2).reshape(P, -1).view(np.uint8)

        blob = np.zeros((P, BLOBW), np.uint8)
        blob[:, :BLOB_ROWL] = rowl8
        blob[:, BLOB_ROWL:BLOB_ROWL + BLOB_SMT] = sm_bytes
        in_maps.append({
            "s": np.ascontiguousarray(sharing_matrix[b, sl]),
            "blob": np.ascontiguousarray(blob),
            "cns": cns,
        })
    return in_maps, n_unique


def kernel(consumption, generation, sharing_matrix, lv_group_ids,
           valid_lv_mask, imbalance_penalty_weight, _want_results=False,
           **run_kwargs):
    w = np.float32(np.asarray(imbalance_penalty_weight))
    in_maps, n_unique = _host_side(consumption, generation, sharing_matrix,
                                   lv_group_ids, valid_lv_mask)
    res = None
    if _want_results or run_kwargs:
        nc = _get_program()
        res = run_bass_kernel_spmd(nc, in_maps, core_ids=list(range(8)),
                                   **run_kwargs)
        parts = np.stack([res.results[c]["out"] for c in range(8)])
    else:
        try:
            fn, in_names, out_names, out_avals = _get_runner()
            concat_in = [np.concatenate([m[name] for m in in_maps], axis=0)
                         for name in in_names]
            zeros = [np.zeros((8 * a.shape[0], *a.shape[1:]), a.dtype)
                     for a in out_avals]
            out_arrs = fn(*concat_in, *zeros)
            parts = np.asarray(out_arrs[out_names.index("out")]).reshape(
                8, P, OW)
        except Exception:
            nc = _get_program()
            res = run_bass_kernel_spmd(nc, in_maps, core_ids=list(range(8)))
            parts = np.stack([res.results[c]["out"] for c in range(8)]).reshape(
                8, P, OW)
    # partition p (< G) carries [gc_p | gg_p | net_p] as 3*T columns
    per_core = parts[:, :G, :].reshape(8, G, 3, T).transpose(0, 2, 1, 3)
    full = per_core.reshape(B, 2, 3, G, T).sum(axis=1, dtype=np.float32)
    gc, gg, net = full[:, 0], full[:, 1], full[:, 2]

    imbalance = np.abs(gc - gg + net)
    total = gc + gg + np.float32(EPS)
    pen = np.maximum(imbalance / total - np.float32(TOL), np.float32(0))
    outv = np.float32(pen.sum(dtype=np.float32) * w / n_unique)
    out_arr = np.array(outv, dtype=np.float32)
    if _want_results:
        return out_arr, res
    return out_arr
